# revision 14
# baseline (speedup 1.0000x reference)
"""Category-specific linear: out[b] = x[b] @ weight[cat[b]] + bias[cat[b]].

Full shapes: x [32, 512, 1024] f32, category_ids [32] int, weight
[64, 1024, 1024] f32, bias [64, 1024] f32 -> out [32, 512, 1024] f32.

Strategy: data-parallel over batch across 8 NeuronCores (4 batches/core).
Host gathers per-batch weights/bias (index-select) and pre-transposes x so
all device DMAs are natural-layout. Each core runs, per batch, a tiled
512x1024x1024 matmul in fp32r (full-rate PE mode for fp32 data) with the
bias added during PSUM eviction on the vector engine.
"""

import numpy as np

import concourse.bass as bass
import concourse.mybir as mybir
from concourse.bass_utils import run_bass_kernel_spmd

# Per-core problem shape
B = 4          # batches per core
L = 512        # rows (seq positions) per batch
K = 1024       # contraction dim
N = 1024       # output dim
KT = K // 128  # 8 k-tiles
LT = L // 128  # 4 l-tiles (output partition tiles)
NT = N // 512  # 2 n-tiles (psum free-dim tiles)
TPB = LT * NT  # 8 output tiles per batch

F32 = mybir.dt.float32
F32R = mybir.dt.float32r


def build_program() -> bass.Bass:
    nc = bass.Bass()

    xt_d = nc.declare_dram_parameter("xt", [B, K, L], F32R, isOutput=False)
    w_d = nc.declare_dram_parameter("w", [B, K, N], F32R, isOutput=False)
    bias_d = nc.declare_dram_parameter("bias", [B, N], F32, isOutput=False)
    out_d = nc.declare_dram_parameter("out", [B, L, N], F32, isOutput=True)

    with (
        nc.sbuf_tensor([128, 2 * KT * L], F32R) as xt_sb,    # 2 bufs x [128, kt*512]
        nc.sbuf_tensor([128, 2 * KT * N], F32R) as w_sb,     # 2 bufs x [128, kt*1024]
        nc.sbuf_tensor([128, 2 * LT * N], F32) as out_sb,    # 2 bufs x [128, lt*1024]
        nc.sbuf_tensor([128, B * N], F32) as bias_bc,
        nc.psum_tensor([128, 4 * 512], F32) as psum,         # 4 banks
        nc.semaphore("s_bias") as s_bias,
        nc.semaphore("s_bc") as s_bc,
        nc.semaphore("s_in") as s_in,
        nc.semaphore("s_out") as s_out,
        nc.semaphore("s_mm") as s_mm,
        nc.semaphore("s_cp") as s_cp,
        nc.Block() as block,
    ):
        XBUF = KT * L    # 4096 floats per buffer in xt_sb
        WBUF = KT * N    # 8192
        OBUF = LT * N    # 4096

        def xt_tile(buf, k, lt):
            # lhsT tile [128(K), 128(L-rows)]
            base = buf * XBUF + k * L + lt * 128
            return xt_sb[:, base : base + 128]

        def w_tile(buf, k, nt):
            # rhs tile [128(K), 512(N)]
            base = buf * WBUF + k * N + nt * 512
            return w_sb[:, base : base + 512]

        @block.sync
        def _(sync):
            # one-time bias load, replicated to all 128 partitions
            sync.dma_start(
                out=bias_bc[:, :],
                in_=bias_d[:, :]
                .rearrange("b n -> (b n)")[None, :]
                .partition_broadcast(128),
            ).then_inc(s_bias, 16)

            def dma_in(b):
                buf = b % 2
                sync.dma_start(
                    out=xt_sb[:, buf * XBUF : (buf + 1) * XBUF].rearrange(
                        "p (kt l) -> p kt l", l=L
                    ),
                    in_=xt_d[b].rearrange("(kt p) l -> p kt l", p=128),
                ).then_inc(s_in, 16)
                sync.dma_start(
                    out=w_sb[:, buf * WBUF : (buf + 1) * WBUF].rearrange(
                        "p (kt n) -> p kt n", n=N
                    ),
                    in_=w_d[b].rearrange("(kt p) n -> p kt n", p=128),
                ).then_inc(s_in, 16)

            def dma_out(b):
                buf = b % 2
                sync.wait_ge(s_cp, (b + 1) * TPB)
                sync.dma_start(
                    out=out_d[b].rearrange("(lt p) n -> p lt n", p=128),
                    in_=out_sb[:, buf * OBUF : (buf + 1) * OBUF].rearrange(
                        "p (lt n) -> p lt n", n=N
                    ),
                ).then_inc(s_out, 16)

            dma_in(0)
            dma_in(1)
            dma_out(0)
            sync.wait_ge(s_mm, 1 * TPB)
            dma_in(2)
            dma_out(1)
            sync.wait_ge(s_mm, 2 * TPB)
            dma_in(3)
            dma_out(2)
            dma_out(3)
            sync.wait_ge(s_out, 4 * 16)
            sync.drain()

        @block.tensor
        def _(tensor):
            tile_idx = 0
            for b in range(B):
                buf = b % 2
                tensor.wait_ge(s_in, 32 * (b + 1))
                for lt in range(LT):
                    for nt in range(NT):
                        bank = tile_idx % 4
                        if tile_idx >= 4:
                            # psum bank must be evicted by DVE first
                            tensor.wait_ge(s_cp, tile_idx - 3)
                        mm = None
                        for k in range(KT):
                            mm = nc.tensor.matmul(
                                psum[:, bank * 512 : (bank + 1) * 512],
                                xt_tile(buf, k, lt),
                                w_tile(buf, k, nt),
                                start=(k == 0),
                                stop=(k == KT - 1),
                            )
                        mm.then_inc(s_mm, 1)
                        tile_idx += 1

        @block.vector
        def _(vector):
            vector.wait_ge(s_bias, 16)
            tile_idx = 0
            for b in range(B):
                buf = b % 2
                if b >= 2:
                    vector.wait_ge(s_out, 16 * (b - 1))
                for lt in range(LT):
                    for nt in range(NT):
                        bank = tile_idx % 4
                        vector.wait_ge(s_mm, tile_idx + 1)
                        nc.vector.tensor_tensor(
                            out=out_sb[
                                :,
                                buf * OBUF + lt * N + nt * 512 : buf * OBUF
                                + lt * N
                                + nt * 512
                                + 512,
                            ],
                            in0=psum[:, bank * 512 : (bank + 1) * 512],
                            in1=bias_bc[:, b * N + nt * 512 : b * N + nt * 512 + 512],
                            op=mybir.AluOpType.add,
                        ).then_inc(s_cp, 1)
                        tile_idx += 1

    return nc


_NC = None


def _get_program():
    global _NC
    if _NC is None:
        _NC = build_program()
    return _NC


def make_in_maps(x, category_ids, weight, bias):
    x = np.asarray(x, dtype=np.float32)
    cids = np.asarray(category_ids).astype(np.int64)
    weight = np.asarray(weight, dtype=np.float32)
    bias = np.asarray(bias, dtype=np.float32)

    wg = weight[cids]                                     # [32, K, N]
    bg = bias[cids]                                       # [32, N]
    xt = np.ascontiguousarray(x.transpose(0, 2, 1))       # [32, K, L]

    in_maps = []
    for c in range(8):
        sl = slice(c * B, (c + 1) * B)
        in_maps.append(
            {
                "xt": np.ascontiguousarray(xt[sl]),
                "w": np.ascontiguousarray(wg[sl]),
                "bias": np.ascontiguousarray(bg[sl]),
            }
        )
    return in_maps


def run_on_device(in_maps, **kwargs):
    return run_bass_kernel_spmd(_get_program(), in_maps, list(range(8)), **kwargs)


def kernel(x, category_ids, weight, bias):
    in_maps = make_in_maps(x, category_ids, weight, bias)
    res = run_on_device(in_maps)
    out = np.concatenate([res.results[c]["out"] for c in range(8)], axis=0)
    return np.ascontiguousarray(out.astype(np.float32))


# revision 26
# speedup vs baseline: 1.0805x; 1.0805x over previous
"""Category-specific linear: out[b] = x[b] @ weight[cat[b]] + bias[cat[b]].

Full shapes: x [32, 512, 1024] f32, category_ids [32] int, weight
[64, 1024, 1024] f32, bias [64, 1024] f32 -> out [32, 512, 1024] f32.

Strategy: data-parallel over batch across 8 NeuronCores (4 batches/core).
Host gathers per-batch weights/bias (index-select) and pre-transposes x so
all device DMAs are natural-layout. Each core runs, per batch, a tiled
512x1024x1024 matmul in fp32r (full-rate PE mode for fp32 data) with the
bias added during PSUM eviction on the vector engine.

Queue split: input DMAs ride the SP (sync) HWDGE ring; output + bias DMAs
ride the ACT (scalar) HWDGE ring, so stores never head-of-line-block loads.
Batch 0 is computed k-outer across all 8 PSUM banks with k-tile-chunked
loads, so the PE starts ~3us in instead of waiting for the full 6MB batch.
"""

import numpy as np

import concourse.bass as bass
import concourse.mybir as mybir
from concourse.bass_utils import run_bass_kernel_spmd

# Per-core problem shape
B = 4          # batches per core
L = 512        # rows (seq positions) per batch
K = 1024       # contraction dim
N = 1024       # output dim
KT = K // 128  # 8 k-tiles
LT = L // 128  # 4 l-tiles (output partition tiles)
NT = N // 512  # 2 n-tiles (psum free-dim tiles)
TPB = LT * NT  # 8 output tiles per batch

F32 = mybir.dt.float32
F32R = mybir.dt.float32r


def build_program() -> bass.Bass:
    nc = bass.Bass()

    xt_d = nc.declare_dram_parameter("xt", [B, K, L], F32R, isOutput=False)
    w_d = nc.declare_dram_parameter("w", [B, K, N], F32R, isOutput=False)
    bias_d = nc.declare_dram_parameter("bias", [B, N], F32, isOutput=False)
    out_d = nc.declare_dram_parameter("out", [B, L, N], F32, isOutput=True)

    from contextlib import ExitStack

    with ExitStack() as ctx:
        xt_sb = ctx.enter_context(nc.sbuf_tensor([128, 2 * KT * L], F32R))
        w_sb = ctx.enter_context(nc.sbuf_tensor([128, 2 * KT * N], F32R))
        out_sb = ctx.enter_context(nc.sbuf_tensor([128, 2 * LT * N], F32))
        bias_bc = ctx.enter_context(nc.sbuf_tensor([128, B * N], F32))
        psum = ctx.enter_context(nc.psum_tensor([128, 8 * 512], F32))  # all 8 banks
        s_bias = ctx.enter_context(nc.semaphore("s_bias"))
        s_chunk = [
            ctx.enter_context(nc.semaphore(f"s_c{k}")) for k in range(KT)
        ]
        s_in = [None] + [
            ctx.enter_context(nc.semaphore(f"s_in{b}")) for b in range(1, B)
        ]
        s_o = [ctx.enter_context(nc.semaphore(f"s_o{b}")) for b in range(B)]
        s_mm = ctx.enter_context(nc.semaphore("s_mm"))
        s_cp = ctx.enter_context(nc.semaphore("s_cp"))
        block = ctx.enter_context(nc.Block())
        XBUF = KT * L    # 4096 floats per buffer in xt_sb
        WBUF = KT * N    # 8192
        OBUF = LT * N    # 4096

        def xt_tile(buf, k, lt):
            # lhsT tile [128(K), 128(L-rows)]
            base = buf * XBUF + k * L + lt * 128
            return xt_sb[:, base : base + 128]

        def w_tile(buf, k, nt):
            # rhs tile [128(K), 512(N)]
            base = buf * WBUF + k * N + nt * 512
            return w_sb[:, base : base + 512]

        def bank(tile_idx):
            return tile_idx % 8

        @block.sync
        def _(sync):
            # batch 0: per-k-tile chunks so the PE can start early
            for k in range(KT):
                sync.dma_start(
                    out=xt_sb[:, k * L : (k + 1) * L],
                    in_=xt_d[0, k * 128 : (k + 1) * 128, :],
                ).then_inc(s_chunk[k], 16)
                sync.dma_start(
                    out=w_sb[:, k * N : (k + 1) * N],
                    in_=w_d[0, k * 128 : (k + 1) * 128, :],
                ).then_inc(s_chunk[k], 16)

            def dma_in(b):
                buf = b % 2
                sync.dma_start(
                    out=xt_sb[:, buf * XBUF : (buf + 1) * XBUF].rearrange(
                        "p (kt l) -> p kt l", l=L
                    ),
                    in_=xt_d[b].rearrange("(kt p) l -> p kt l", p=128),
                ).then_inc(s_in[b], 16)
                sync.dma_start(
                    out=w_sb[:, buf * WBUF : (buf + 1) * WBUF].rearrange(
                        "p (kt n) -> p kt n", n=N
                    ),
                    in_=w_d[b].rearrange("(kt p) n -> p kt n", p=128),
                ).then_inc(s_in[b], 16)

            dma_in(1)
            sync.wait_ge(s_mm, 1 * TPB)  # PE done reading buf0 (batch 0)
            dma_in(2)
            sync.wait_ge(s_mm, 2 * TPB)
            dma_in(3)
            for b in range(B):
                sync.wait_ge(s_o[b], 16)
            sync.drain()

        @block.scalar
        def _(scalar):
            # bias load, replicated to all 128 partitions
            scalar.dma_start(
                out=bias_bc[:, :],
                in_=bias_d[:, :]
                .rearrange("b n -> (b n)")[None, :]
                .partition_broadcast(128),
            ).then_inc(s_bias, 16)

            for b in range(B):
                buf = b % 2
                scalar.wait_ge(s_cp, (b + 1) * TPB)
                scalar.dma_start(
                    out=out_d[b].rearrange("(lt p) n -> p lt n", p=128),
                    in_=out_sb[:, buf * OBUF : (buf + 1) * OBUF].rearrange(
                        "p (lt n) -> p lt n", n=N
                    ),
                ).then_inc(s_o[b], 16)

        @block.tensor
        def _(tensor):
            # batch 0: k-outer over all 8 psum banks, chunk-granular waits
            for k in range(KT):
                tensor.wait_ge(s_chunk[k], 32)
                for lt in range(LT):
                    for nt in range(NT):
                        t = lt * NT + nt
                        mm = nc.tensor.matmul(
                            psum[:, t * 512 : (t + 1) * 512],
                            xt_tile(0, k, lt),
                            w_tile(0, k, nt),
                            start=(k == 0),
                            stop=(k == KT - 1),
                        )
                        if k == KT - 1:
                            mm.then_inc(s_mm, 1)

            # batches 1..3: tile-outer, k-inner
            for b in range(1, B):
                buf = b % 2
                tensor.wait_ge(s_in[b], 32)
                for lt in range(LT):
                    for nt in range(NT):
                        tile_idx = b * TPB + lt * NT + nt
                        # psum bank must have been evicted by DVE
                        tensor.wait_ge(s_cp, tile_idx - 7)
                        mm = None
                        for k in range(KT):
                            mm = nc.tensor.matmul(
                                psum[:, bank(tile_idx) * 512 : (bank(tile_idx) + 1) * 512],
                                xt_tile(buf, k, lt),
                                w_tile(buf, k, nt),
                                start=(k == 0),
                                stop=(k == KT - 1),
                            )
                        mm.then_inc(s_mm, 1)

        @block.vector
        def _(vector):
            vector.wait_ge(s_bias, 16)
            for b in range(B):
                buf = b % 2
                if b >= 2:
                    vector.wait_ge(s_o[b - 2], 16)
                for lt in range(LT):
                    for nt in range(NT):
                        tile_idx = b * TPB + lt * NT + nt
                        vector.wait_ge(s_mm, tile_idx + 1)
                        nc.vector.tensor_tensor(
                            out=out_sb[
                                :,
                                buf * OBUF + lt * N + nt * 512 : buf * OBUF
                                + lt * N
                                + nt * 512
                                + 512,
                            ],
                            in0=psum[:, bank(tile_idx) * 512 : (bank(tile_idx) + 1) * 512],
                            in1=bias_bc[:, b * N + nt * 512 : b * N + nt * 512 + 512],
                            op=mybir.AluOpType.add,
                        ).then_inc(s_cp, 1)

    return nc


_NC = None


def _get_program():
    global _NC
    if _NC is None:
        _NC = build_program()
    return _NC


def make_in_maps(x, category_ids, weight, bias):
    x = np.asarray(x, dtype=np.float32)
    cids = np.asarray(category_ids).astype(np.int64)
    weight = np.asarray(weight, dtype=np.float32)
    bias = np.asarray(bias, dtype=np.float32)

    wg = weight[cids]                                     # [32, K, N]
    bg = bias[cids]                                       # [32, N]
    xt = np.ascontiguousarray(x.transpose(0, 2, 1))       # [32, K, L]

    in_maps = []
    for c in range(8):
        sl = slice(c * B, (c + 1) * B)
        in_maps.append(
            {
                "xt": np.ascontiguousarray(xt[sl]),
                "w": np.ascontiguousarray(wg[sl]),
                "bias": np.ascontiguousarray(bg[sl]),
            }
        )
    return in_maps


def run_on_device(in_maps, **kwargs):
    return run_bass_kernel_spmd(_get_program(), in_maps, list(range(8)), **kwargs)


def kernel(x, category_ids, weight, bias):
    in_maps = make_in_maps(x, category_ids, weight, bias)
    res = run_on_device(in_maps)
    out = np.concatenate([res.results[c]["out"] for c in range(8)], axis=0)
    return np.ascontiguousarray(out.astype(np.float32))


# revision 28
# speedup vs baseline: 1.1128x; 1.0299x over previous
"""Category-specific linear: out[b] = x[b] @ weight[cat[b]] + bias[cat[b]].

Full shapes: x [32, 512, 1024] f32, category_ids [32] int, weight
[64, 1024, 1024] f32, bias [64, 1024] f32 -> out [32, 512, 1024] f32.

Strategy: data-parallel over batch across 8 NeuronCores (4 batches/core).
Host gathers per-batch weights/bias (index-select) and pre-transposes x so
all device DMAs are natural-layout. Each core runs, per batch, a tiled
512x1024x1024 matmul in fp32r (full-rate PE mode for fp32 data).

Pipeline: every batch is computed k-outer across all 8 PSUM banks with
k-chunked loads, so the PE trails the DMA stream by ~one chunk. The bias
is folded into the matmul as a K=1 accumulation term (ones[1,128].T @
bias[1,512]), so PSUM eviction is a plain vector copy. Input DMAs ride
the SP HWDGE ring; output + constant DMAs ride the ACT ring, so stores
never head-of-line-block loads. Outputs drain in half-batch chunks.
"""

from contextlib import ExitStack

import numpy as np

import concourse.bass as bass
import concourse.mybir as mybir
from concourse.bass_utils import run_bass_kernel_spmd

# Per-core problem shape
B = 4           # batches per core
L = 512         # rows (seq positions) per batch
K = 1024        # contraction dim
N = 1024        # output dim
KT = K // 128   # 8 k-tiles
LT = L // 128   # 4 l-tiles (output partition tiles)
NT = N // 512   # 2 n-tiles (psum free-dim tiles)
TPB = LT * NT   # 8 output tiles per batch = 8 psum banks
NCHUNK = 4      # input chunks per batch (2 k-tiles each)
KPC = KT // NCHUNK  # k-tiles per chunk

F32 = mybir.dt.float32
F32R = mybir.dt.float32r


def build_program() -> bass.Bass:
    nc = bass.Bass()

    xt_d = nc.declare_dram_parameter("xt", [B, K, L], F32R, isOutput=False)
    w_d = nc.declare_dram_parameter("w", [B, K, N], F32R, isOutput=False)
    bias_d = nc.declare_dram_parameter("bias", [B, N], F32R, isOutput=False)
    ones_d = nc.declare_dram_parameter("ones", [1, 128], F32R, isOutput=False)
    out_d = nc.declare_dram_parameter("out", [B, L, N], F32, isOutput=True)

    with ExitStack() as ctx:
        xt_sb = ctx.enter_context(nc.sbuf_tensor([128, 2 * KT * L], F32R))
        w_sb = ctx.enter_context(nc.sbuf_tensor([128, 2 * KT * N], F32R))
        out_sb = ctx.enter_context(nc.sbuf_tensor([128, 2 * LT * N], F32))
        bias_sb = ctx.enter_context(nc.sbuf_tensor([1, B * N], F32R))
        ones_sb = ctx.enter_context(nc.sbuf_tensor([1, 128], F32R))
        psum = ctx.enter_context(nc.psum_tensor([128, 8 * 512], F32))  # 8 banks
        s_const = ctx.enter_context(nc.semaphore("s_const"))
        s_chunk = [ctx.enter_context(nc.semaphore(f"s_c{c}")) for c in range(NCHUNK)]
        s_o = [ctx.enter_context(nc.semaphore(f"s_o{b}")) for b in range(B)]
        s_mm = ctx.enter_context(nc.semaphore("s_mm"))
        s_cp = ctx.enter_context(nc.semaphore("s_cp"))
        block = ctx.enter_context(nc.Block())

        XBUF = KT * L    # 4096 floats per buffer in xt_sb
        WBUF = KT * N    # 8192
        OBUF = LT * N    # 4096

        def xt_tile(buf, k, lt):
            # lhsT tile [128(K), 128(L-rows)]
            base = buf * XBUF + k * L + lt * 128
            return xt_sb[:, base : base + 128]

        def w_tile(buf, k, nt):
            # rhs tile [128(K), 512(N)]
            base = buf * WBUF + k * N + nt * 512
            return w_sb[:, base : base + 512]

        @block.sync
        def _(sync):
            for b in range(B):
                buf = b % 2
                if b >= 2:
                    # chunks overwrite the buffer batch b-2 was reading
                    sync.wait_ge(s_mm, (b - 1) * TPB)
                for c in range(NCHUNK):
                    k0 = c * KPC
                    sync.dma_start(
                        out=xt_sb[
                            :, buf * XBUF + k0 * L : buf * XBUF + (k0 + KPC) * L
                        ].rearrange("p (kt l) -> p kt l", l=L),
                        in_=xt_d[b, k0 * 128 : (k0 + KPC) * 128, :].rearrange(
                            "(kt p) l -> p kt l", p=128
                        ),
                    ).then_inc(s_chunk[c], 16)
                    sync.dma_start(
                        out=w_sb[
                            :, buf * WBUF + k0 * N : buf * WBUF + (k0 + KPC) * N
                        ].rearrange("p (kt n) -> p kt n", n=N),
                        in_=w_d[b, k0 * 128 : (k0 + KPC) * 128, :].rearrange(
                            "(kt p) n -> p kt n", p=128
                        ),
                    ).then_inc(s_chunk[c], 16)
            for b in range(B):
                sync.wait_ge(s_o[b], 32)
            sync.drain()

        @block.scalar
        def _(scalar):
            scalar.dma_start(
                out=bias_sb[:, :],
                in_=bias_d[:, :].rearrange("b n -> (b n)")[None, :],
            ).then_inc(s_const, 16)
            scalar.dma_start(out=ones_sb[:, :], in_=ones_d[:, :]).then_inc(s_const, 16)

            for b in range(B):
                buf = b % 2
                for h in range(2):  # half-batch output chunks (2 lt each)
                    scalar.wait_ge(s_cp, b * TPB + (h + 1) * 4)
                    scalar.dma_start(
                        out=out_d[b, h * 256 : (h + 1) * 256, :].rearrange(
                            "(lt p) n -> p lt n", p=128
                        ),
                        in_=out_sb[
                            :,
                            buf * OBUF + h * 2048 : buf * OBUF + (h + 1) * 2048,
                        ].rearrange("p (lt n) -> p lt n", n=N),
                    ).then_inc(s_o[b], 16)

        @block.tensor
        def _(tensor):
            waited_const = False
            for b in range(B):
                buf = b % 2
                for k in range(KT):
                    if k % KPC == 0:
                        tensor.wait_ge(s_chunk[k // KPC], 32 * (b + 1))
                    for t in range(TPB):
                        lt, nt = divmod(t, NT)
                        if k == 0 and b > 0:
                            # bank t must have been evicted from batch b-1
                            tensor.wait_ge(s_cp, (b - 1) * TPB + t + 1)
                        nc.tensor.matmul(
                            psum[:, t * 512 : (t + 1) * 512],
                            xt_tile(buf, k, lt),
                            w_tile(buf, k, nt),
                            start=(k == 0),
                            stop=False,
                        )
                # bias fold-in: psum[t] += ones[1,128].T @ bias[1,512]
                if not waited_const:
                    tensor.wait_ge(s_const, 32)
                    waited_const = True
                for t in range(TPB):
                    lt, nt = divmod(t, NT)
                    nc.tensor.matmul(
                        psum[:, t * 512 : (t + 1) * 512],
                        ones_sb[0:1, :],
                        bias_sb[0:1, b * N + nt * 512 : b * N + nt * 512 + 512],
                        start=False,
                        stop=True,
                    ).then_inc(s_mm, 1)

        @block.vector
        def _(vector):
            for b in range(B):
                buf = b % 2
                if b >= 2:
                    vector.wait_ge(s_o[b - 2], 32)
                for t in range(TPB):
                    lt, nt = divmod(t, NT)
                    vector.wait_ge(s_mm, b * TPB + t + 1)
                    nc.vector.tensor_copy(
                        out=out_sb[
                            :,
                            buf * OBUF + lt * N + nt * 512 : buf * OBUF
                            + lt * N
                            + nt * 512
                            + 512,
                        ],
                        in_=psum[:, t * 512 : (t + 1) * 512],
                    ).then_inc(s_cp, 1)

    return nc


_NC = None


def _get_program():
    global _NC
    if _NC is None:
        _NC = build_program()
    return _NC


def make_in_maps(x, category_ids, weight, bias):
    x = np.asarray(x, dtype=np.float32)
    cids = np.asarray(category_ids).astype(np.int64)
    weight = np.asarray(weight, dtype=np.float32)
    bias = np.asarray(bias, dtype=np.float32)

    wg = weight[cids]                                     # [32, K, N]
    bg = bias[cids]                                       # [32, N]
    xt = np.ascontiguousarray(x.transpose(0, 2, 1))       # [32, K, L]
    ones = np.ones((1, 128), dtype=np.float32)

    in_maps = []
    for c in range(8):
        sl = slice(c * B, (c + 1) * B)
        in_maps.append(
            {
                "xt": np.ascontiguousarray(xt[sl]),
                "w": np.ascontiguousarray(wg[sl]),
                "bias": np.ascontiguousarray(bg[sl]),
                "ones": ones,
            }
        )
    return in_maps


def run_on_device(in_maps, **kwargs):
    return run_bass_kernel_spmd(_get_program(), in_maps, list(range(8)), **kwargs)


def kernel(x, category_ids, weight, bias):
    in_maps = make_in_maps(x, category_ids, weight, bias)
    res = run_on_device(in_maps)
    out = np.concatenate([res.results[c]["out"] for c in range(8)], axis=0)
    return np.ascontiguousarray(out.astype(np.float32))


# revision 30
# speedup vs baseline: 1.2771x; 1.1477x over previous
"""Category-specific linear: out[b] = x[b] @ weight[cat[b]] + bias[cat[b]].

Full shapes: x [32, 512, 1024] f32, category_ids [32] int, weight
[64, 1024, 1024] f32, bias [64, 1024] f32 -> out [32, 512, 1024] f32.

Strategy: data-parallel over batch across 8 NeuronCores (4 batches/core).
Host gathers per-batch weights/bias (index-select) and pre-transposes x so
all device DMAs are natural-layout. Each core runs, per batch, a tiled
512x1024x1024 matmul in fp32r (full-rate PE mode for fp32 data).

Pipeline: every batch is computed k-outer across all 8 PSUM banks with
per-k-tile chunked loads (triple-buffered), so the PE trails the DMA
stream by ~one k-tile and never idles long enough to drop out of the
HAM fast clock. The bias is folded into the matmul as a K=1 accumulation
term (ones[1,128].T @ bias[1,512]), so PSUM eviction is a plain vector
copy. Input DMAs ride the SP HWDGE ring; output + constant DMAs ride the
ACT ring, so stores never head-of-line-block loads. Outputs drain in
quarter-batch chunks to shorten the tail.
"""

from contextlib import ExitStack

import numpy as np

import concourse.bass as bass
import concourse.mybir as mybir
from concourse.bass_utils import run_bass_kernel_spmd

# Per-core problem shape
B = 4           # batches per core
L = 512         # rows (seq positions) per batch
K = 1024        # contraction dim
N = 1024        # output dim
KT = K // 128   # 8 k-tiles = 8 input chunks per batch
LT = L // 128   # 4 l-tiles (output partition tiles)
NT = N // 512   # 2 n-tiles (psum free-dim tiles)
TPB = LT * NT   # 8 output tiles per batch = 8 psum banks
NBUF = 3        # input buffers
OCH = 4         # output chunks per batch (2 tiles each)

F32 = mybir.dt.float32
F32R = mybir.dt.float32r


def build_program() -> bass.Bass:
    nc = bass.Bass()

    xt_d = nc.declare_dram_parameter("xt", [B, K, L], F32R, isOutput=False)
    w_d = nc.declare_dram_parameter("w", [B, K, N], F32R, isOutput=False)
    bias_d = nc.declare_dram_parameter("bias", [B, N], F32R, isOutput=False)
    ones_d = nc.declare_dram_parameter("ones", [1, 128], F32R, isOutput=False)
    out_d = nc.declare_dram_parameter("out", [B, L, N], F32, isOutput=True)

    with ExitStack() as ctx:
        xt_sb = ctx.enter_context(nc.sbuf_tensor([128, NBUF * KT * L], F32R))
        w_sb = ctx.enter_context(nc.sbuf_tensor([128, NBUF * KT * N], F32R))
        out_sb = ctx.enter_context(nc.sbuf_tensor([128, 2 * LT * N], F32))
        bias_sb = ctx.enter_context(nc.sbuf_tensor([1, B * N], F32R))
        ones_sb = ctx.enter_context(nc.sbuf_tensor([1, 128], F32R))
        psum = ctx.enter_context(nc.psum_tensor([128, 8 * 512], F32))  # 8 banks
        s_const = ctx.enter_context(nc.semaphore("s_const"))
        s_chunk = [ctx.enter_context(nc.semaphore(f"s_c{c}")) for c in range(KT)]
        s_o = [ctx.enter_context(nc.semaphore(f"s_o{b}")) for b in range(B)]
        s_mm = ctx.enter_context(nc.semaphore("s_mm"))
        s_cp = ctx.enter_context(nc.semaphore("s_cp"))
        block = ctx.enter_context(nc.Block())

        XBUF = KT * L    # 4096 floats per buffer in xt_sb
        WBUF = KT * N    # 8192
        OBUF = LT * N    # 4096

        def xt_tile(buf, k, lt):
            # lhsT tile [128(K), 128(L-rows)]
            base = buf * XBUF + k * L + lt * 128
            return xt_sb[:, base : base + 128]

        def w_tile(buf, k, nt):
            # rhs tile [128(K), 512(N)]
            base = buf * WBUF + k * N + nt * 512
            return w_sb[:, base : base + 512]

        @block.sync
        def _(sync):
            for b in range(B):
                buf = b % NBUF
                if b >= NBUF:
                    # chunks overwrite the buffer batch b-NBUF was reading
                    sync.wait_ge(s_mm, (b - NBUF + 1) * TPB)
                for k in range(KT):
                    sync.dma_start(
                        out=xt_sb[:, buf * XBUF + k * L : buf * XBUF + (k + 1) * L],
                        in_=xt_d[b, k * 128 : (k + 1) * 128, :],
                    ).then_inc(s_chunk[k], 16)
                    sync.dma_start(
                        out=w_sb[:, buf * WBUF + k * N : buf * WBUF + (k + 1) * N],
                        in_=w_d[b, k * 128 : (k + 1) * 128, :],
                    ).then_inc(s_chunk[k], 16)
            for b in range(B):
                sync.wait_ge(s_o[b], OCH * 16)
            sync.drain()

        @block.scalar
        def _(scalar):
            scalar.dma_start(
                out=bias_sb[:, :],
                in_=bias_d[:, :].rearrange("b n -> (b n)")[None, :],
            ).then_inc(s_const, 16)
            scalar.dma_start(out=ones_sb[:, :], in_=ones_d[:, :]).then_inc(s_const, 16)

            TPO = TPB // OCH  # tiles per output chunk = 2
            for b in range(B):
                obuf = b % 2
                for h in range(OCH):
                    # chunk h = l-tile h: tiles (h*NT .. h*NT+NT-1), rows
                    # h*128..(h+1)*128, full N
                    scalar.wait_ge(s_cp, b * TPB + (h + 1) * TPO)
                    scalar.dma_start(
                        out=out_d[b, h * 128 : (h + 1) * 128, :],
                        in_=out_sb[:, obuf * OBUF + h * N : obuf * OBUF + (h + 1) * N],
                    ).then_inc(s_o[b], 16)

        @block.tensor
        def _(tensor):
            waited_const = False
            for b in range(B):
                buf = b % NBUF
                for k in range(KT):
                    tensor.wait_ge(s_chunk[k], 32 * (b + 1))
                    for t in range(TPB):
                        lt, nt = divmod(t, NT)
                        if k == 0 and b > 0:
                            # bank t must have been evicted from batch b-1
                            tensor.wait_ge(s_cp, (b - 1) * TPB + t + 1)
                        nc.tensor.matmul(
                            psum[:, t * 512 : (t + 1) * 512],
                            xt_tile(buf, k, lt),
                            w_tile(buf, k, nt),
                            start=(k == 0),
                            stop=False,
                        )
                # bias fold-in: psum[t] += ones[1,128].T @ bias[1,512]
                if not waited_const:
                    tensor.wait_ge(s_const, 32)
                    waited_const = True
                for t in range(TPB):
                    lt, nt = divmod(t, NT)
                    nc.tensor.matmul(
                        psum[:, t * 512 : (t + 1) * 512],
                        ones_sb[0:1, :],
                        bias_sb[0:1, b * N + nt * 512 : b * N + nt * 512 + 512],
                        start=False,
                        stop=True,
                    ).then_inc(s_mm, 1)

        @block.vector
        def _(vector):
            for b in range(B):
                obuf = b % 2
                if b >= 2:
                    vector.wait_ge(s_o[b - 2], OCH * 16)
                for t in range(TPB):
                    lt, nt = divmod(t, NT)
                    vector.wait_ge(s_mm, b * TPB + t + 1)
                    nc.vector.tensor_copy(
                        out=out_sb[
                            :,
                            obuf * OBUF + lt * N + nt * 512 : obuf * OBUF
                            + lt * N
                            + nt * 512
                            + 512,
                        ],
                        in_=psum[:, t * 512 : (t + 1) * 512],
                    ).then_inc(s_cp, 1)

    return nc


_NC = None


def _get_program():
    global _NC
    if _NC is None:
        _NC = build_program()
    return _NC


def make_in_maps(x, category_ids, weight, bias):
    x = np.asarray(x, dtype=np.float32)
    cids = np.asarray(category_ids).astype(np.int64)
    weight = np.asarray(weight, dtype=np.float32)
    bias = np.asarray(bias, dtype=np.float32)

    wg = weight[cids]                                     # [32, K, N]
    bg = bias[cids]                                       # [32, N]
    xt = np.ascontiguousarray(x.transpose(0, 2, 1))       # [32, K, L]
    ones = np.ones((1, 128), dtype=np.float32)

    in_maps = []
    for c in range(8):
        sl = slice(c * B, (c + 1) * B)
        in_maps.append(
            {
                "xt": np.ascontiguousarray(xt[sl]),
                "w": np.ascontiguousarray(wg[sl]),
                "bias": np.ascontiguousarray(bg[sl]),
                "ones": ones,
            }
        )
    return in_maps


def run_on_device(in_maps, **kwargs):
    return run_bass_kernel_spmd(_get_program(), in_maps, list(range(8)), **kwargs)


def kernel(x, category_ids, weight, bias):
    in_maps = make_in_maps(x, category_ids, weight, bias)
    res = run_on_device(in_maps)
    out = np.concatenate([res.results[c]["out"] for c in range(8)], axis=0)
    return np.ascontiguousarray(out.astype(np.float32))


# revision 31
# speedup vs baseline: 1.2891x; 1.0094x over previous
"""Category-specific linear: out[b] = x[b] @ weight[cat[b]] + bias[cat[b]].

Full shapes: x [32, 512, 1024] f32, category_ids [32] int, weight
[64, 1024, 1024] f32, bias [64, 1024] f32 -> out [32, 512, 1024] f32.

Strategy: data-parallel over batch across 8 NeuronCores (4 batches/core).
Host gathers per-batch weights/bias (index-select) and pre-transposes x so
all device DMAs are natural-layout. Each core runs, per batch, a tiled
512x1024x1024 matmul in fp32r (full-rate PE mode for fp32 data).

Pipeline: every batch is computed k-outer across all 8 PSUM banks with
per-k-tile chunked loads (triple-buffered), so the PE trails the DMA
stream by ~one k-tile and never idles long enough to drop out of the
HAM fast clock. The bias is folded into the matmul as a K=1 accumulation
term (ones[1,128].T @ bias[1,512]), so PSUM eviction is a plain vector
copy. Input DMAs ride the SP HWDGE ring; output + constant DMAs ride the
ACT ring, so stores never head-of-line-block loads. Outputs drain in
quarter-batch chunks to shorten the tail.
"""

from contextlib import ExitStack

import numpy as np

import concourse.bass as bass
import concourse.mybir as mybir
from concourse.bass_utils import run_bass_kernel_spmd

# Per-core problem shape
B = 4           # batches per core
L = 512         # rows (seq positions) per batch
K = 1024        # contraction dim
N = 1024        # output dim
KT = K // 128   # 8 k-tiles = 8 input chunks per batch
LT = L // 128   # 4 l-tiles (output partition tiles)
NT = N // 512   # 2 n-tiles (psum free-dim tiles)
TPB = LT * NT   # 8 output tiles per batch = 8 psum banks
NBUF = 3        # input buffers
OCH = 4         # output chunks per batch (2 tiles each)

F32 = mybir.dt.float32
F32R = mybir.dt.float32r


def build_program() -> bass.Bass:
    nc = bass.Bass()

    xt_d = nc.declare_dram_parameter("xt", [B, K, L], F32R, isOutput=False)
    w_d = nc.declare_dram_parameter("w", [B, K, N], F32R, isOutput=False)
    bias_d = nc.declare_dram_parameter("bias", [B, N], F32R, isOutput=False)
    ones_d = nc.declare_dram_parameter("ones", [1, 128], F32R, isOutput=False)
    out_d = nc.declare_dram_parameter("out", [B, L, N], F32, isOutput=True)

    with ExitStack() as ctx:
        xt_sb = ctx.enter_context(nc.sbuf_tensor([128, NBUF * KT * L], F32R))
        w_sb = ctx.enter_context(nc.sbuf_tensor([128, NBUF * KT * N], F32R))
        out_sb = ctx.enter_context(nc.sbuf_tensor([128, 2 * LT * N], F32))
        bias_sb = ctx.enter_context(nc.sbuf_tensor([1, B * N], F32R))
        ones_sb = ctx.enter_context(nc.sbuf_tensor([1, 128], F32R))
        psum = ctx.enter_context(nc.psum_tensor([128, 8 * 512], F32))  # 8 banks
        s_const = ctx.enter_context(nc.semaphore("s_const"))
        s_chunk = [ctx.enter_context(nc.semaphore(f"s_c{c}")) for c in range(KT)]
        s_o = [ctx.enter_context(nc.semaphore(f"s_o{b}")) for b in range(B)]
        s_mm = ctx.enter_context(nc.semaphore("s_mm"))
        s_cp = ctx.enter_context(nc.semaphore("s_cp"))
        block = ctx.enter_context(nc.Block())

        XBUF = KT * L    # 4096 floats per buffer in xt_sb
        WBUF = KT * N    # 8192
        OBUF = LT * N    # 4096

        def xt_tile(buf, k, lt):
            # lhsT tile [128(K), 128(L-rows)]
            base = buf * XBUF + k * L + lt * 128
            return xt_sb[:, base : base + 128]

        def w_tile(buf, k, nt):
            # rhs tile [128(K), 512(N)]
            base = buf * WBUF + k * N + nt * 512
            return w_sb[:, base : base + 512]

        @block.sync
        def _(sync):
            for b in range(B):
                buf = b % NBUF
                if b >= NBUF:
                    # chunks overwrite the buffer batch b-NBUF was reading
                    sync.wait_ge(s_mm, (b - NBUF + 1) * TPB)
                for k in range(KT):
                    sync.dma_start(
                        out=xt_sb[:, buf * XBUF + k * L : buf * XBUF + (k + 1) * L],
                        in_=xt_d[b, k * 128 : (k + 1) * 128, :],
                    ).then_inc(s_chunk[k], 16)
                    sync.dma_start(
                        out=w_sb[:, buf * WBUF + k * N : buf * WBUF + (k + 1) * N],
                        in_=w_d[b, k * 128 : (k + 1) * 128, :],
                    ).then_inc(s_chunk[k], 16)
            for b in range(B):
                sync.wait_ge(s_o[b], OCH * 16)
            sync.drain()

        @block.scalar
        def _(scalar):
            scalar.dma_start(
                out=bias_sb[:, :],
                in_=bias_d[:, :].rearrange("b n -> (b n)")[None, :],
            ).then_inc(s_const, 16)
            scalar.dma_start(out=ones_sb[:, :], in_=ones_d[:, :]).then_inc(s_const, 16)

            TPO = TPB // OCH  # tiles per output chunk = 2
            for b in range(B):
                obuf = b % 2
                for h in range(OCH):
                    # chunk h = l-tile h: tiles (h*NT .. h*NT+NT-1), rows
                    # h*128..(h+1)*128, full N
                    scalar.wait_ge(s_cp, b * TPB + (h + 1) * TPO)
                    scalar.dma_start(
                        out=out_d[b, h * 128 : (h + 1) * 128, :],
                        in_=out_sb[:, obuf * OBUF + h * N : obuf * OBUF + (h + 1) * N],
                    ).then_inc(s_o[b], 16)

        @block.tensor
        def _(tensor):
            tensor.wait_ge(s_const, 32)
            for b in range(B):
                buf = b % NBUF
                # bias first: psum[t] = ones[1,128].T @ bias[1,512], so the
                # accumulation group ends on k7 and the batch tail is short
                for t in range(TPB):
                    lt, nt = divmod(t, NT)
                    if b > 0:
                        # bank t must have been evicted from batch b-1
                        tensor.wait_ge(s_cp, (b - 1) * TPB + t + 1)
                    nc.tensor.matmul(
                        psum[:, t * 512 : (t + 1) * 512],
                        ones_sb[0:1, :],
                        bias_sb[0:1, b * N + nt * 512 : b * N + nt * 512 + 512],
                        start=True,
                        stop=False,
                    )
                for k in range(KT):
                    tensor.wait_ge(s_chunk[k], 32 * (b + 1))
                    for t in range(TPB):
                        lt, nt = divmod(t, NT)
                        mm = nc.tensor.matmul(
                            psum[:, t * 512 : (t + 1) * 512],
                            xt_tile(buf, k, lt),
                            w_tile(buf, k, nt),
                            start=False,
                            stop=(k == KT - 1),
                        )
                        if k == KT - 1:
                            mm.then_inc(s_mm, 1)

        @block.vector
        def _(vector):
            for b in range(B):
                obuf = b % 2
                if b >= 2:
                    vector.wait_ge(s_o[b - 2], OCH * 16)
                for t in range(TPB):
                    lt, nt = divmod(t, NT)
                    vector.wait_ge(s_mm, b * TPB + t + 1)
                    nc.vector.tensor_copy(
                        out=out_sb[
                            :,
                            obuf * OBUF + lt * N + nt * 512 : obuf * OBUF
                            + lt * N
                            + nt * 512
                            + 512,
                        ],
                        in_=psum[:, t * 512 : (t + 1) * 512],
                    ).then_inc(s_cp, 1)

    return nc


_NC = None


def _get_program():
    global _NC
    if _NC is None:
        _NC = build_program()
    return _NC


def make_in_maps(x, category_ids, weight, bias):
    x = np.asarray(x, dtype=np.float32)
    cids = np.asarray(category_ids).astype(np.int64)
    weight = np.asarray(weight, dtype=np.float32)
    bias = np.asarray(bias, dtype=np.float32)

    wg = weight[cids]                                     # [32, K, N]
    bg = bias[cids]                                       # [32, N]
    xt = np.ascontiguousarray(x.transpose(0, 2, 1))       # [32, K, L]
    ones = np.ones((1, 128), dtype=np.float32)

    in_maps = []
    for c in range(8):
        sl = slice(c * B, (c + 1) * B)
        in_maps.append(
            {
                "xt": np.ascontiguousarray(xt[sl]),
                "w": np.ascontiguousarray(wg[sl]),
                "bias": np.ascontiguousarray(bg[sl]),
                "ones": ones,
            }
        )
    return in_maps


def run_on_device(in_maps, **kwargs):
    return run_bass_kernel_spmd(_get_program(), in_maps, list(range(8)), **kwargs)


def kernel(x, category_ids, weight, bias):
    in_maps = make_in_maps(x, category_ids, weight, bias)
    res = run_on_device(in_maps)
    out = np.concatenate([res.results[c]["out"] for c in range(8)], axis=0)
    return np.ascontiguousarray(out.astype(np.float32))


# revision 33
# speedup vs baseline: 1.3287x; 1.0307x over previous
"""Category-specific linear: out[b] = x[b] @ weight[cat[b]] + bias[cat[b]].

Full shapes: x [32, 512, 1024] f32, category_ids [32] int, weight
[64, 1024, 1024] f32, bias [64, 1024] f32 -> out [32, 512, 1024] f32.

Strategy: data-parallel over batch across 8 NeuronCores (4 batches/core).
Host gathers per-batch weights/bias (index-select) and pre-transposes x so
all device DMAs are natural-layout. Each core runs, per batch, a tiled
512x1024x1024 matmul in fp32r (full-rate PE mode for fp32 data).

Pipeline: every batch is computed k-outer across all 8 PSUM banks with
per-k-tile chunked loads (triple-buffered), so the PE trails the DMA
stream by ~one k-tile and never idles long enough to drop out of the
HAM fast clock. The bias is folded into the matmul as a K=1 accumulation
term (ones[1,128].T @ bias[1,512]), so PSUM eviction is a plain vector
copy. Input DMAs ride the SP HWDGE ring; output + constant DMAs ride the
ACT ring, so stores never head-of-line-block loads. Outputs drain in
quarter-batch chunks to shorten the tail.
"""

from contextlib import ExitStack

import numpy as np

import concourse.bass as bass
import concourse.mybir as mybir
from concourse.bass_utils import run_bass_kernel_spmd

# Per-core problem shape
B = 4           # batches per core
L = 512         # rows (seq positions) per batch
K = 1024        # contraction dim
N = 1024        # output dim
KT = K // 128   # 8 k-tiles = 8 input chunks per batch
LT = L // 128   # 4 l-tiles (output partition tiles)
NT = N // 512   # 2 n-tiles (psum free-dim tiles)
TPB = LT * NT   # 8 output tiles per batch = 8 psum banks
NBUF = 3        # input buffers
OCH = 4         # output chunks per batch (2 tiles each)

F32 = mybir.dt.float32
F32R = mybir.dt.float32r

# matmul input dtype: float32r is fp32 data at full PE rate; float16/bfloat16
# halve the HBM stream at reduced precision
IN_DT = F32R


def build_program(in_dt=None) -> bass.Bass:
    if in_dt is None:
        in_dt = IN_DT
    nc = bass.Bass()

    xt_d = nc.declare_dram_parameter("xt", [B, K, L], in_dt, isOutput=False)
    w_d = nc.declare_dram_parameter("w", [B, K, N], in_dt, isOutput=False)
    bias_d = nc.declare_dram_parameter("bias", [B, N], in_dt, isOutput=False)
    ones_d = nc.declare_dram_parameter("ones", [1, 128], in_dt, isOutput=False)
    out_d = nc.declare_dram_parameter("out", [B, L, N], F32, isOutput=True)

    with ExitStack() as ctx:
        xt_sb = ctx.enter_context(nc.sbuf_tensor([128, NBUF * KT * L], in_dt))
        w_sb = ctx.enter_context(nc.sbuf_tensor([128, NBUF * KT * N], in_dt))
        out_sb = ctx.enter_context(nc.sbuf_tensor([128, 2 * LT * N], F32))
        bias_sb = ctx.enter_context(nc.sbuf_tensor([1, B * N], in_dt))
        ones_sb = ctx.enter_context(nc.sbuf_tensor([1, 128], in_dt))
        psum = ctx.enter_context(nc.psum_tensor([128, 8 * 512], F32))  # 8 banks
        s_const = ctx.enter_context(nc.semaphore("s_const"))
        s_chunk = [ctx.enter_context(nc.semaphore(f"s_c{c}")) for c in range(KT)]
        s_o = [ctx.enter_context(nc.semaphore(f"s_o{b}")) for b in range(B)]
        s_mm = ctx.enter_context(nc.semaphore("s_mm"))
        s_cp = ctx.enter_context(nc.semaphore("s_cp"))
        block = ctx.enter_context(nc.Block())

        XBUF = KT * L    # 4096 floats per buffer in xt_sb
        WBUF = KT * N    # 8192
        OBUF = LT * N    # 4096

        def xt_tile(buf, k, lt):
            # lhsT tile [128(K), 128(L-rows)]
            base = buf * XBUF + k * L + lt * 128
            return xt_sb[:, base : base + 128]

        def w_tile(buf, k, nt):
            # rhs tile [128(K), 512(N)]
            base = buf * WBUF + k * N + nt * 512
            return w_sb[:, base : base + 512]

        @block.sync
        def _(sync):
            for b in range(B):
                buf = b % NBUF
                if b >= NBUF:
                    # chunks overwrite the buffer batch b-NBUF was reading
                    sync.wait_ge(s_mm, (b - NBUF + 1) * TPB)
                for k in range(KT):
                    sync.dma_start(
                        out=xt_sb[:, buf * XBUF + k * L : buf * XBUF + (k + 1) * L],
                        in_=xt_d[b, k * 128 : (k + 1) * 128, :],
                    ).then_inc(s_chunk[k], 16)
                    sync.dma_start(
                        out=w_sb[:, buf * WBUF + k * N : buf * WBUF + (k + 1) * N],
                        in_=w_d[b, k * 128 : (k + 1) * 128, :],
                    ).then_inc(s_chunk[k], 16)
            for b in range(B):
                sync.wait_ge(s_o[b], OCH * 16)
            sync.drain()

        @block.scalar
        def _(scalar):
            scalar.dma_start(
                out=bias_sb[:, :],
                in_=bias_d[:, :].rearrange("b n -> (b n)")[None, :],
            ).then_inc(s_const, 16)
            scalar.dma_start(out=ones_sb[:, :], in_=ones_d[:, :]).then_inc(s_const, 16)

            TPO = TPB // OCH  # tiles per output chunk = 2
            for b in range(B):
                obuf = b % 2
                for h in range(OCH):
                    # chunk h = l-tile h: tiles (h*NT .. h*NT+NT-1), rows
                    # h*128..(h+1)*128, full N
                    scalar.wait_ge(s_cp, b * TPB + (h + 1) * TPO)
                    scalar.dma_start(
                        out=out_d[b, h * 128 : (h + 1) * 128, :],
                        in_=out_sb[:, obuf * OBUF + h * N : obuf * OBUF + (h + 1) * N],
                    ).then_inc(s_o[b], 16)

        @block.tensor
        def _(tensor):
            tensor.wait_ge(s_const, 32)
            for b in range(B):
                buf = b % NBUF
                # bias first: psum[t] = ones[1,128].T @ bias[1,512], so the
                # accumulation group ends on k7 and the batch tail is short
                for t in range(TPB):
                    lt, nt = divmod(t, NT)
                    if b > 0:
                        # bank t must have been evicted from batch b-1
                        tensor.wait_ge(s_cp, (b - 1) * TPB + t + 1)
                    nc.tensor.matmul(
                        psum[:, t * 512 : (t + 1) * 512],
                        ones_sb[0:1, :],
                        bias_sb[0:1, b * N + nt * 512 : b * N + nt * 512 + 512],
                        start=True,
                        stop=False,
                    )
                for k in range(KT):
                    tensor.wait_ge(s_chunk[k], 32 * (b + 1))
                    for t in range(TPB):
                        lt, nt = divmod(t, NT)
                        mm = nc.tensor.matmul(
                            psum[:, t * 512 : (t + 1) * 512],
                            xt_tile(buf, k, lt),
                            w_tile(buf, k, nt),
                            start=False,
                            stop=(k == KT - 1),
                        )
                        if k == KT - 1:
                            mm.then_inc(s_mm, 1)

        @block.vector
        def _(vector):
            for b in range(B):
                obuf = b % 2
                if b >= 2:
                    vector.wait_ge(s_o[b - 2], OCH * 16)
                for t in range(TPB):
                    lt, nt = divmod(t, NT)
                    vector.wait_ge(s_mm, b * TPB + t + 1)
                    nc.vector.tensor_copy(
                        out=out_sb[
                            :,
                            obuf * OBUF + lt * N + nt * 512 : obuf * OBUF
                            + lt * N
                            + nt * 512
                            + 512,
                        ],
                        in_=psum[:, t * 512 : (t + 1) * 512],
                    ).then_inc(s_cp, 1)

    return nc


_NC = None


def _get_program():
    global _NC
    if _NC is None:
        _NC = build_program()
    return _NC


def make_in_maps(x, category_ids, weight, bias, np_dt=np.float32):
    x = np.asarray(x, dtype=np.float32)
    cids = np.asarray(category_ids).astype(np.int64)
    weight = np.asarray(weight, dtype=np.float32)
    bias = np.asarray(bias, dtype=np.float32)

    wg = weight[cids].astype(np_dt)                       # [32, K, N]
    bg = bias[cids].astype(np_dt)                         # [32, N]
    xt = np.ascontiguousarray(x.transpose(0, 2, 1)).astype(np_dt)  # [32, K, L]
    ones = np.ones((1, 128), dtype=np_dt)

    in_maps = []
    for c in range(8):
        sl = slice(c * B, (c + 1) * B)
        in_maps.append(
            {
                "xt": np.ascontiguousarray(xt[sl]),
                "w": np.ascontiguousarray(wg[sl]),
                "bias": np.ascontiguousarray(bg[sl]),
                "ones": ones,
            }
        )
    return in_maps


def run_on_device(in_maps, **kwargs):
    return run_bass_kernel_spmd(_get_program(), in_maps, list(range(8)), **kwargs)


def kernel(x, category_ids, weight, bias):
    in_maps = make_in_maps(x, category_ids, weight, bias)
    res = run_on_device(in_maps)
    out = np.concatenate([res.results[c]["out"] for c in range(8)], axis=0)
    return np.ascontiguousarray(out.astype(np.float32))
